# revision 11
# baseline (speedup 1.0000x reference)
"""Trainium2 Bass kernel for nn_FC_CharNet (dense_mlp).

Strategy (pure data-parallel with a host-glued exchange):
  Launch 1 (phase A): batch dim B=128 sharded 8 ways (16 rows/core).
    Each core runs the three stage-1 MLP stacks (board 6075->1024->256->64,
    order 40->128->32->64, message 20->64->32->64) for its 16*10*8 = 1280
    rows, entirely in "transposed activation" form (channels on partitions,
    rows on the free dim) so no on-device transposes are needed. Outputs
    pre-BN features [64, 1280] per stream.
  Host glue: BatchNorm training-mode stats over the full batch + relu +
    the reference's batch-scrambling reshape ([P,C,B,3S] -> [P,S,B,H]),
    then reshard by LSTM row.
  Launch 2 (phase B): LSTM rows i' sharded 8 ways (16 rows x 10 pasts =
    160 columns/core). 8-step LSTM + head, all in transposed form.

Matmuls run in bf16 (fp32 PSUM accumulation); rel err vs the fp32
reference lands around 1e-3.
"""

import numpy as np
import ml_dtypes

import concourse.bass as bass
import concourse.tile as tile
from concourse import bacc, mybir
from concourse.bass_utils import run_bass_kernel_spmd

BF16 = ml_dtypes.bfloat16
F32 = np.float32

# Problem shape (hardcoded per contract)
B, P, S = 128, 10, 8
L, BFEAT = 81, 75
OF, MF2 = 40, 20
C = 64
H = 3 * C  # 192
NCHAR = 8
EPS = 1e-5

N_CORES = 8
BS = B // N_CORES          # 16 batch rows per core (phase A)
ROWS = BS * P * S          # 1280 rows per core (phase A)
KB = L * BFEAT             # 6075
KBP = 6144                 # padded to 48*128
NKC = KBP // 128           # 48 k-chunks
Q = 4                      # quarters of the row dim
QC = ROWS // Q             # 320 columns per quarter

IS = B // N_CORES          # 16 LSTM rows per core (phase B)
NB = IS * P                # 160 LSTM columns per core

_DT = mybir.dt


def _ceil_div(a, b):
    return (a + b - 1) // b


# ---------------------------------------------------------------------------
# Phase A kernel build
# ---------------------------------------------------------------------------

def _build_phase_a():
    nc = bacc.Bacc("TRN2", target_bir_lowering=False, debug=False,
                   num_devices=N_CORES)

    def din(name, shape, dt=_DT.bfloat16):
        return nc.dram_tensor(name, shape, dt, kind="ExternalInput").ap()

    def dout(name, shape, dt=_DT.float32):
        return nc.dram_tensor(name, shape, dt, kind="ExternalOutput").ap()

    xT = din("xT", [NKC, 128, ROWS])            # board.T, k-chunked
    ordT = din("ordT", [OF, ROWS])
    msgT = din("msgT", [MF2, ROWS])
    w1T = din("w1T", [NKC, 128, 1024])
    w2T = din("w2T", [8, 128, 256])
    w3T = din("w3T", [2, 128, C])
    wo1T = din("wo1T", [OF, 128])
    wo2T = din("wo2T", [128, 32])
    wo3T = din("wo3T", [32, C])
    wm1T = din("wm1T", [MF2, 64])
    wm2T = din("wm2T", [64, 32])
    wm3T = din("wm3T", [32, C])
    bb1 = din("bb1", [1024], _DT.float32)
    bb2 = din("bb2", [256], _DT.float32)
    bb3 = din("bb3", [C], _DT.float32)
    bo1 = din("bo1", [128], _DT.float32)
    bo2 = din("bo2", [32], _DT.float32)
    bo3 = din("bo3", [C], _DT.float32)
    bm1 = din("bm1", [64], _DT.float32)
    bm2 = din("bm2", [32], _DT.float32)
    bm3 = din("bm3", [C], _DT.float32)

    out_b = dout("out_b", [C, ROWS])
    out_o = dout("out_o", [C, ROWS])
    out_m = dout("out_m", [C, ROWS])

    RELU = mybir.ActivationFunctionType.Relu
    IDENT = mybir.ActivationFunctionType.Identity

    with tile.TileContext(nc) as tc:
        with (
            tc.tile_pool(name="wpool", bufs=1) as wpool,
            tc.tile_pool(name="xpool", bufs=2) as xpool,
            tc.tile_pool(name="hpool", bufs=2) as hpool,
            tc.tile_pool(name="opool", bufs=2) as opool,
            tc.tile_pool(name="psA", bufs=4, space="PSUM") as psA,
            tc.tile_pool(name="psB", bufs=2, space="PSUM") as psB,
            tc.tile_pool(name="cpool", bufs=1) as cpool,
        ):
            # --- resident weights / constants -----------------------------
            w1 = wpool.tile([128, NKC, 1024], _DT.bfloat16)
            for g in range(8):  # 8 DMAs of 6 k-chunks each
                nc.sync.dma_start(
                    out=w1[:, g * 6:(g + 1) * 6, :],
                    in_=xTsrc(w1T, g * 6, 6, 0, 1024))
            w2 = wpool.tile([128, 8, 256], _DT.bfloat16)
            nc.sync.dma_start(out=w2, in_=w2T.rearrange("k p n -> p k n"))
            w3 = wpool.tile([128, 2, C], _DT.bfloat16)
            nc.sync.dma_start(out=w3, in_=w3T.rearrange("k p n -> p k n"))

            wo1 = cpool.tile([OF, 128], _DT.bfloat16)
            nc.sync.dma_start(out=wo1, in_=wo1T)
            wo2 = cpool.tile([128, 32], _DT.bfloat16)
            nc.sync.dma_start(out=wo2, in_=wo2T)
            wo3 = cpool.tile([32, C], _DT.bfloat16)
            nc.sync.dma_start(out=wo3, in_=wo3T)
            wm1 = cpool.tile([MF2, 64], _DT.bfloat16)
            nc.sync.dma_start(out=wm1, in_=wm1T)
            wm2 = cpool.tile([64, 32], _DT.bfloat16)
            nc.sync.dma_start(out=wm2, in_=wm2T)
            wm3 = cpool.tile([32, C], _DT.bfloat16)
            nc.sync.dma_start(out=wm3, in_=wm3T)

            ordt = cpool.tile([OF, ROWS], _DT.bfloat16)
            nc.sync.dma_start(out=ordt, in_=ordT)
            msgt = cpool.tile([MF2, ROWS], _DT.bfloat16)
            nc.sync.dma_start(out=msgt, in_=msgT)

            bb1s = cpool.tile([128, 8], _DT.float32)
            nc.sync.dma_start(out=bb1s, in_=bb1.rearrange("(m p) -> p m", p=128))
            bb2s = cpool.tile([128, 2], _DT.float32)
            nc.sync.dma_start(out=bb2s, in_=bb2.rearrange("(m p) -> p m", p=128))
            bb3s = cpool.tile([C, 1], _DT.float32)
            nc.sync.dma_start(out=bb3s, in_=bb3.unsqueeze(1))
            bo1s = cpool.tile([128, 1], _DT.float32)
            nc.sync.dma_start(out=bo1s, in_=bo1.unsqueeze(1))
            bo2s = cpool.tile([32, 1], _DT.float32)
            nc.sync.dma_start(out=bo2s, in_=bo2.unsqueeze(1))
            bo3s = cpool.tile([C, 1], _DT.float32)
            nc.sync.dma_start(out=bo3s, in_=bo3.unsqueeze(1))
            bm1s = cpool.tile([64, 1], _DT.float32)
            nc.sync.dma_start(out=bm1s, in_=bm1.unsqueeze(1))
            bm2s = cpool.tile([32, 1], _DT.float32)
            nc.sync.dma_start(out=bm2s, in_=bm2.unsqueeze(1))
            bm3s = cpool.tile([C, 1], _DT.float32)
            nc.sync.dma_start(out=bm3s, in_=bm3.unsqueeze(1))

            # --- main loop over row quarters ------------------------------
            for q in range(Q):
                xt = xpool.tile([128, NKC, QC], _DT.bfloat16)
                for g in range(4):  # 4 DMAs of 12 k-chunks
                    nc.sync.dma_start(
                        out=xt[:, g * 12:(g + 1) * 12, :],
                        in_=xTsrc(xT, g * 12, 12, q * QC, QC))

                # board stage 1: 6144 -> 1024
                h1 = hpool.tile([128, 8, QC], _DT.bfloat16, tag="h1")
                for m in range(8):
                    ps = psA.tile([128, QC], _DT.float32, tag="ps1")
                    for k in range(NKC):
                        nc.tensor.matmul(
                            ps, w1[:, k, m * 128:(m + 1) * 128], xt[:, k, :],
                            start=(k == 0), stop=(k == NKC - 1))
                    nc.scalar.activation(h1[:, m, :], ps, RELU,
                                         bias=bb1s[:, m:m + 1])

                # board stage 2: 1024 -> 256
                h2 = hpool.tile([128, 2, QC], _DT.bfloat16, tag="h2")
                for m in range(2):
                    ps = psA.tile([128, QC], _DT.float32, tag="ps1")
                    for k in range(8):
                        nc.tensor.matmul(
                            ps, w2[:, k, m * 128:(m + 1) * 128], h1[:, k, :],
                            start=(k == 0), stop=(k == 7))
                    nc.scalar.activation(h2[:, m, :], ps, RELU,
                                         bias=bb2s[:, m:m + 1])

                # board stage 3: 256 -> 64 (linear)
                ps3 = psB.tile([C, QC], _DT.float32, tag="ps3")
                for k in range(2):
                    nc.tensor.matmul(ps3, w3[:, k, :], h2[:, k, :],
                                     start=(k == 0), stop=(k == 1))
                ob = opool.tile([C, QC], _DT.float32, tag="ob")
                nc.scalar.activation(ob, ps3, IDENT, bias=bb3s[:, 0:1])
                nc.sync.dma_start(out=out_b[:, q * QC:(q + 1) * QC], in_=ob)

                # order MLP: 40 -> 128 -> 32 -> 64
                pso1 = psA.tile([128, QC], _DT.float32, tag="ps1")
                nc.tensor.matmul(pso1, wo1, ordt[:, q * QC:(q + 1) * QC],
                                 start=True, stop=True)
                oh1 = hpool.tile([128, QC], _DT.bfloat16, tag="oh1")
                nc.scalar.activation(oh1, pso1, RELU, bias=bo1s[:, 0:1])
                pso2 = psB.tile([32, QC], _DT.float32, tag="ps3")
                nc.tensor.matmul(pso2, wo2, oh1, start=True, stop=True)
                oh2 = hpool.tile([32, QC], _DT.bfloat16, tag="oh2")
                nc.scalar.activation(oh2, pso2, RELU, bias=bo2s[:, 0:1])
                pso3 = psB.tile([C, QC], _DT.float32, tag="ps3")
                nc.tensor.matmul(pso3, wo3, oh2, start=True, stop=True)
                oo = opool.tile([C, QC], _DT.float32, tag="oo")
                nc.scalar.activation(oo, pso3, IDENT, bias=bo3s[:, 0:1])
                nc.sync.dma_start(out=out_o[:, q * QC:(q + 1) * QC], in_=oo)

                # message MLP: 20 -> 64 -> 32 -> 64
                psm1 = psB.tile([64, QC], _DT.float32, tag="ps3")
                nc.tensor.matmul(psm1, wm1, msgt[:, q * QC:(q + 1) * QC],
                                 start=True, stop=True)
                mh1 = hpool.tile([64, QC], _DT.bfloat16, tag="mh1")
                nc.scalar.activation(mh1, psm1, RELU, bias=bm1s[:, 0:1])
                psm2 = psB.tile([32, QC], _DT.float32, tag="ps3")
                nc.tensor.matmul(psm2, wm2, mh1, start=True, stop=True)
                mh2 = hpool.tile([32, QC], _DT.bfloat16, tag="mh2")
                nc.scalar.activation(mh2, psm2, RELU, bias=bm2s[:, 0:1])
                psm3 = psB.tile([C, QC], _DT.float32, tag="ps3")
                nc.tensor.matmul(psm3, wm3, mh2, start=True, stop=True)
                om = opool.tile([C, QC], _DT.float32, tag="om")
                nc.scalar.activation(om, psm3, IDENT, bias=bm3s[:, 0:1])
                nc.sync.dma_start(out=out_m[:, q * QC:(q + 1) * QC], in_=om)

    nc.compile()
    return nc


def xTsrc(ap, k0, nk, c0, ncols):
    """[NKC, 128, COLS] dram AP -> [128, nk, ncols] slice in SBUF dim order."""
    return ap[k0:k0 + nk, :, c0:c0 + ncols].rearrange("k p n -> p k n")


# ---------------------------------------------------------------------------
# Host-side prep / glue
# ---------------------------------------------------------------------------

def _prep_phase_a(inputs):
    """Build per-core in_maps for phase A."""
    board = np.asarray(inputs["board"], F32)
    order = np.asarray(inputs["order"], F32)
    message = np.asarray(inputs["message"], F32)

    shared = {
        "w1T": _padT(np.asarray(inputs["Wb1"], F32), KBP).reshape(NKC, 128, 1024),
        "w2T": np.ascontiguousarray(np.asarray(inputs["Wb2"], F32).T).astype(BF16).reshape(8, 128, 256),
        "w3T": np.ascontiguousarray(np.asarray(inputs["Wb3"], F32).T).astype(BF16).reshape(2, 128, C),
        "wo1T": np.ascontiguousarray(np.asarray(inputs["Wo1"], F32).T).astype(BF16),
        "wo2T": np.ascontiguousarray(np.asarray(inputs["Wo2"], F32).T).astype(BF16),
        "wo3T": np.ascontiguousarray(np.asarray(inputs["Wo3"], F32).T).astype(BF16),
        "wm1T": np.ascontiguousarray(np.asarray(inputs["Wm1"], F32).T).astype(BF16),
        "wm2T": np.ascontiguousarray(np.asarray(inputs["Wm2"], F32).T).astype(BF16),
        "wm3T": np.ascontiguousarray(np.asarray(inputs["Wm3"], F32).T).astype(BF16),
    }
    for k in ("bb1", "bb2", "bb3", "bo1", "bo2", "bo3", "bm1", "bm2", "bm3"):
        pass
    shared["bb1"] = np.asarray(inputs["bb1"], F32)
    shared["bb2"] = np.asarray(inputs["bb2"], F32)
    shared["bb3"] = np.asarray(inputs["bb3"], F32)
    shared["bo1"] = np.asarray(inputs["bo1"], F32)
    shared["bo2"] = np.asarray(inputs["bo2"], F32)
    shared["bo3"] = np.asarray(inputs["bo3"], F32)
    shared["bm1"] = np.asarray(inputs["bm1"], F32)
    shared["bm2"] = np.asarray(inputs["bm2"], F32)
    shared["bm3"] = np.asarray(inputs["bm3"], F32)

    in_maps = []
    for c in range(N_CORES):
        sl = slice(c * BS, (c + 1) * BS)
        xT = _padT(board[sl].reshape(ROWS, KB), KBP).reshape(NKC, 128, ROWS)
        ordT = np.ascontiguousarray(order[sl].reshape(ROWS, OF).T).astype(BF16)
        msgT = np.ascontiguousarray(message[sl].reshape(ROWS, MF2).T).astype(BF16)
        m = {"xT": xT, "ordT": ordT, "msgT": msgT}
        m.update(shared)
        in_maps.append(m)
    return in_maps


def _padT(a, kpad):
    """[rows, k] fp32 -> transposed, k zero-padded, bf16 [kpad, rows]."""
    rows, k = a.shape
    out = np.zeros((kpad, rows), BF16)
    out[:k] = np.ascontiguousarray(a.T).astype(BF16)
    return out


def _glue(outs_a, inputs):
    """Device pre-BN outputs -> BN + relu + scramble -> phase-B xf.

    outs_a: list of 8 per-core dicts with out_b/out_o/out_m [64, 1280].
    Returns xf [P, S, B, H] float32."""
    streams = []
    for key in ("out_b", "out_o", "out_m"):
        # [C, rows] per core, rows = (b_local, p, s) -> full [B, P, S, C]
        full = np.stack([outs_a[c][key].reshape(C, BS, P, S)
                         for c in range(N_CORES)], axis=1)  # [C, 8, 16, P, S]
        streams.append(full.reshape(C, B, P, S))
    g = [np.asarray(inputs["g1"], F32), np.asarray(inputs["g2"], F32),
         np.asarray(inputs["g3"], F32)]
    be = [np.asarray(inputs["be1"], F32), np.asarray(inputs["be2"], F32),
          np.asarray(inputs["be3"], F32)]
    ys = []
    for x, gg, bb in zip(streams, g, be):
        # x: [C, B, P, S]; BN per (p, c) over (b, s)
        m = x.mean(axis=(1, 3), keepdims=True)
        v = ((x - m) ** 2).mean(axis=(1, 3), keepdims=True)
        y = np.maximum(gg[:, None, None, None] * (x - m) / np.sqrt(v + EPS)
                       + bb[:, None, None, None], 0.0)
        ys.append(np.transpose(y, (2, 0, 1, 3)))  # [P, C, B, S]
    xfeat = np.concatenate(ys, axis=3)  # [P, C, B, 3S]
    return xfeat.reshape(P, S, B, H)  # torch-bug scramble


# ---------------------------------------------------------------------------
# Phase B kernel build
# ---------------------------------------------------------------------------

def _build_phase_b():
    nc = bacc.Bacc("TRN2", target_bir_lowering=False, debug=False,
                   num_devices=N_CORES)

    def din(name, shape, dt=_DT.bfloat16):
        return nc.dram_tensor(name, shape, dt, kind="ExternalInput").ap()

    xTs = din("xTs", [S, H, NB])          # scrambled lstm inputs
    h0T = din("h0T", [H, NB])
    c0T = din("c0T", [H, NB], _DT.float32)
    # [Wih; Whh].T with gate columns permuted to
    # [i(0:128) f(0:128) g(0:128) o(0:128) | i(128:192) f g o] so that each
    # gate lands partition-aligned with the h/c state slabs.
    wT = din("wT", [2 * H, 4 * H])
    bihh = din("bihh", [4 * H], _DT.float32)  # same permutation, chunked
    wf1T = din("wf1T", [S * H, 256])
    bf1 = din("bf1", [256], _DT.float32)
    wf2T = din("wf2T", [270, NCHAR])
    bf2 = din("bf2", [NCHAR], _DT.float32)
    oimiT = din("oimiT", [14, NB])

    out = nc.dram_tensor("out", [NCHAR, NB], _DT.float32,
                         kind="ExternalOutput").ap()

    RELU = mybir.ActivationFunctionType.Relu
    IDENT = mybir.ActivationFunctionType.Identity
    SIG = mybir.ActivationFunctionType.Sigmoid
    TANH = mybir.ActivationFunctionType.Tanh

    with tile.TileContext(nc) as tc:
        with (
            tc.tile_pool(name="const", bufs=1) as const,
            tc.tile_pool(name="xp", bufs=3) as xp,
            tc.tile_pool(name="gp", bufs=2) as gp,
            tc.tile_pool(name="cp", bufs=2) as cp,
            tc.tile_pool(name="ps", bufs=3, space="PSUM") as ps,
            tc.tile_pool(name="pso", bufs=1, space="PSUM") as pso,
        ):
            # weights: wT k-chunks [0:128],[128:192],[192:320],[320:384]
            wta = const.tile([128, 4 * H], _DT.bfloat16)
            nc.sync.dma_start(out=wta, in_=wT[0:128, :])
            wtb = const.tile([64, 4 * H], _DT.bfloat16)
            nc.sync.dma_start(out=wtb, in_=wT[128:192, :])
            wtc = const.tile([128, 4 * H], _DT.bfloat16)
            nc.sync.dma_start(out=wtc, in_=wT[192:320, :])
            wtd = const.tile([64, 4 * H], _DT.bfloat16)
            nc.sync.dma_start(out=wtd, in_=wT[320:384, :])
            # bias: first 512 = ab-slab chunks [128, 4]; last 256 = c-slab
            # chunks [64, 4]
            bgab = const.tile([128, 4], _DT.float32)
            nc.sync.dma_start(out=bgab,
                              in_=bihh[0:512].rearrange("(m p) -> p m", p=128))
            bgc = const.tile([64, 4], _DT.float32)
            nc.sync.dma_start(out=bgc,
                              in_=bihh[512:768].rearrange("(m p) -> p m", p=64))

            wf1a = const.tile([128, S, 256], _DT.bfloat16)
            wf1b = const.tile([64, S, 256], _DT.bfloat16)
            for s in range(S):
                nc.sync.dma_start(out=wf1a[:, s, :],
                                  in_=wf1T[s * H:s * H + 128, :])
                nc.sync.dma_start(out=wf1b[:, s, :],
                                  in_=wf1T[s * H + 128:(s + 1) * H, :])
            bf1s = const.tile([128, 2], _DT.float32)
            nc.sync.dma_start(out=bf1s, in_=bf1.rearrange("(m p) -> p m", p=128))
            wf2a = const.tile([128, NCHAR], _DT.bfloat16)
            nc.sync.dma_start(out=wf2a, in_=wf2T[0:128, :])
            wf2b = const.tile([128, NCHAR], _DT.bfloat16)
            nc.sync.dma_start(out=wf2b, in_=wf2T[128:256, :])
            wf2c = const.tile([14, NCHAR], _DT.bfloat16)
            nc.sync.dma_start(out=wf2c, in_=wf2T[256:270, :])
            bf2s = const.tile([NCHAR, 1], _DT.float32)
            nc.sync.dma_start(out=bf2s, in_=bf2.unsqueeze(1))
            oimi = const.tile([14, NB], _DT.bfloat16)
            nc.sync.dma_start(out=oimi, in_=oimiT)

            # lstm state
            h0a = const.tile([128, NB], _DT.bfloat16)
            nc.sync.dma_start(out=h0a, in_=h0T[0:128, :])
            h0b = const.tile([64, NB], _DT.bfloat16)
            nc.sync.dma_start(out=h0b, in_=h0T[128:H, :])
            c_a = const.tile([128, NB], _DT.float32)  # rows 0:128 of c
            nc.sync.dma_start(out=c_a, in_=c0T[0:128, :])
            c_b = const.tile([64, NB], _DT.float32)   # rows 128:192
            nc.sync.dma_start(out=c_b, in_=c0T[128:H, :])

            # h stack (also the recurrent state feed)
            hsa = const.tile([128, S, NB], _DT.bfloat16)
            hsb = const.tile([64, S, NB], _DT.bfloat16)

            GATE_FUNCS = (SIG, SIG, TANH, SIG)  # i, f, g, o
            for s in range(S):
                xt0 = xp.tile([128, NB], _DT.bfloat16, tag="xt0")
                nc.sync.dma_start(out=xt0, in_=xTs[s, 0:128, :])
                xt1 = xp.tile([64, NB], _DT.bfloat16, tag="xt1")
                nc.sync.dma_start(out=xt1, in_=xTs[s, 128:H, :])
                hp0 = h0a if s == 0 else hsa[:, s - 1, :]
                hp1 = h0b if s == 0 else hsb[:, s - 1, :]

                # gates: ab slab = 4 chunks of M=128 (wT cols 0:512),
                #        c slab  = 4 chunks of M=64  (wT cols 512:768)
                gab = gp.tile([128, 4, NB], _DT.float32, tag="gab")
                gc = gp.tile([64, 4, NB], _DT.float32, tag="gc")
                for g in range(4):
                    pg = ps.tile([128, NB], _DT.float32, tag="pab")
                    sl = slice(g * 128, (g + 1) * 128)
                    nc.tensor.matmul(pg, wta[:, sl], xt0, start=True, stop=False)
                    nc.tensor.matmul(pg, wtb[:, sl], xt1, start=False, stop=False)
                    nc.tensor.matmul(pg, wtc[:, sl], hp0, start=False, stop=False)
                    nc.tensor.matmul(pg, wtd[:, sl], hp1, start=False, stop=True)
                    nc.scalar.activation(gab[:, g, :], pg, GATE_FUNCS[g],
                                         bias=bgab[:, g:g + 1])
                    pc = ps.tile([64, NB], _DT.float32, tag="pc")
                    slc = slice(512 + g * 64, 512 + (g + 1) * 64)
                    nc.tensor.matmul(pc, wta[:, slc], xt0, start=True, stop=False)
                    nc.tensor.matmul(pc, wtb[:, slc], xt1, start=False, stop=False)
                    nc.tensor.matmul(pc, wtc[:, slc], hp0, start=False, stop=False)
                    nc.tensor.matmul(pc, wtd[:, slc], hp1, start=False, stop=True)
                    nc.scalar.activation(gc[:, g, :], pc, GATE_FUNCS[g],
                                         bias=bgc[:, g:g + 1])

                # cell update: ab slab (128 rows), c slab (64 rows)
                for gt, ct, hout, nrow in (
                    (gab, c_a, hsa[:, s, :], 128),
                    (gc, c_b, hsb[:, s, :], 64),
                ):
                    t1 = cp.tile([nrow, NB], _DT.float32, tag=f"t1{nrow}")
                    nc.vector.tensor_mul(t1, gt[:, 1, :], ct)       # f * c
                    t2 = cp.tile([nrow, NB], _DT.float32, tag=f"t2{nrow}")
                    nc.vector.tensor_mul(t2, gt[:, 0, :], gt[:, 2, :])  # i * g
                    nc.vector.tensor_add(ct, t1, t2)
                    tct = cp.tile([nrow, NB], _DT.float32, tag=f"tc{nrow}")
                    nc.scalar.activation(tct, ct, TANH)
                    nc.vector.tensor_mul(hout, gt[:, 3, :], tct)    # o * tanh(c)

            # head: x2 = relu(Wf1 @ hstack + bf1)
            x2 = gp.tile([128, 2, NB], _DT.bfloat16, tag="x2")
            for m in range(2):
                pg = ps.tile([128, NB], _DT.float32, tag="pab")
                sl = slice(m * 128, (m + 1) * 128)
                for s in range(S):
                    nc.tensor.matmul(pg, wf1a[:, s, sl], hsa[:, s, :],
                                     start=(s == 0), stop=False)
                    nc.tensor.matmul(pg, wf1b[:, s, sl], hsb[:, s, :],
                                     start=False, stop=(s == S - 1))
                nc.scalar.activation(x2[:, m, :], pg, RELU,
                                     bias=bf1s[:, m:m + 1])

            po = pso.tile([NCHAR, NB], _DT.float32, tag="po")
            nc.tensor.matmul(po, wf2a, x2[:, 0, :], start=True, stop=False)
            nc.tensor.matmul(po, wf2b, x2[:, 1, :], start=False, stop=False)
            nc.tensor.matmul(po, wf2c, oimi, start=False, stop=True)
            osb = gp.tile([NCHAR, NB], _DT.float32, tag="osb")
            nc.scalar.activation(osb, po, IDENT, bias=bf2s[:, 0:1])
            nc.sync.dma_start(out=out, in_=osb)

    nc.compile()
    return nc


def _prep_phase_b(xf, inputs):
    """xf [P, S, B, H] -> per-core phase-B in_maps. Column order is
    n = i_local * P + p."""
    h0 = np.asarray(inputs["h0"], F32)  # [P, B, H]
    c0 = np.asarray(inputs["c0"], F32)
    oi = np.asarray(inputs["other_ind"], F32)  # [B, P, 7]
    mi = np.asarray(inputs["me_ind"], F32)

    # gate permutation: [i(0:128) f g o | i(128:192) f g o] (see kernel)
    perm = np.concatenate([np.arange(g * 192, g * 192 + 128) for g in range(4)]
                          + [np.arange(g * 192 + 128, (g + 1) * 192)
                             for g in range(4)])
    wihh = np.concatenate([np.asarray(inputs["Wih"], F32),
                           np.asarray(inputs["Whh"], F32)], axis=1)[perm]
    wT = np.ascontiguousarray(wihh.T).astype(BF16)
    shared = {
        "wT": wT,
        "bihh": (np.asarray(inputs["bih"], F32)
                 + np.asarray(inputs["bhh"], F32))[perm],
        "wf1T": np.ascontiguousarray(np.asarray(inputs["Wf1"], F32).T).astype(BF16),
        "bf1": np.asarray(inputs["bf1"], F32),
        "wf2T": np.ascontiguousarray(np.asarray(inputs["Wf2"], F32).T).astype(BF16),
        "bf2": np.asarray(inputs["bf2"], F32),
    }

    in_maps = []
    for c in range(N_CORES):
        rows = slice(c * IS, (c + 1) * IS)
        # xTs: [S, H, IS, P] with column order (i_local, p)
        x = xf[:, :, rows, :]  # [P, S, IS, H]
        xTs = np.ascontiguousarray(
            np.transpose(x, (1, 3, 2, 0))).astype(BF16).reshape(S, H, NB)
        h0T = np.ascontiguousarray(
            np.transpose(h0[:, rows, :], (2, 1, 0))).reshape(H, NB)
        c0T = np.ascontiguousarray(
            np.transpose(c0[:, rows, :], (2, 1, 0))).reshape(H, NB)
        oimi = np.concatenate([oi[rows], mi[rows]], axis=2)  # [IS, P, 14]
        oimiT = np.ascontiguousarray(
            np.transpose(oimi, (2, 0, 1))).reshape(14, NB)
        m = {
            "xTs": xTs,
            "h0T": h0T.astype(BF16),
            "c0T": c0T.astype(F32),
            "oimiT": oimiT.astype(BF16),
        }
        m.update(shared)
        in_maps.append(m)
    return in_maps


def _finish(outs_b):
    """Per-core [NCHAR, NB] -> [B, NCHAR] (sum over p)."""
    res = np.zeros((B, NCHAR), F32)
    for c in range(N_CORES):
        o = outs_b[c]["out"].reshape(NCHAR, IS, P)
        res[c * IS:(c + 1) * IS] = o.sum(axis=2).T
    return res


# ---------------------------------------------------------------------------
# Entry point
# ---------------------------------------------------------------------------

_NC_A = None
_NC_B = None


def _get_kernels():
    global _NC_A, _NC_B
    if _NC_A is None:
        _NC_A = _build_phase_a()
    if _NC_B is None:
        _NC_B = _build_phase_b()
    return _NC_A, _NC_B


def kernel(**inputs):
    nca, ncb = _get_kernels()
    in_a = _prep_phase_a(inputs)
    res_a = run_bass_kernel_spmd(nca, in_a, core_ids=list(range(N_CORES)))
    xf = _glue(res_a.results, inputs)
    in_b = _prep_phase_b(xf, inputs)
    res_b = run_bass_kernel_spmd(ncb, in_b, core_ids=list(range(N_CORES)))
    return _finish(res_b.results)
